# revision 7
# baseline (speedup 1.0000x reference)
"""Graphormer attention (N=2048, D=512, H=8 heads of 64) on 8 NeuronCores.

Strategy (tensor-parallel over heads, one head per core):
  - Host slices Q/K/V/O projection weights per head, transposes x once.
  - The z-bin bias is folded in multiplicatively: the per-head bias table is
    tiny (16 entries), so host precomputes W = exp(z_table[bin(z)]) transposed
    to the kernel's [key, query] layout, shipped as bf16.
  - On device (per core): Q^T/K^T/V projections, S^T = K^T-tiles x Q^T
    (fp32, PSUM), exp on ScalarE -> bf16, P = exp(S) * W on VectorE,
    O'^T = sum_k V'[k,65] x P (65th V column = ones => row 64 of O' is the
    softmax denominator Z), then Y^T = Wo_h^T-tiles x O^T.
  - Host divides each head's partial Y by its Z, sums heads, adds bias terms.
"""

import numpy as np
import ml_dtypes
from contextlib import ExitStack

import concourse.bass as bass
import concourse.tile as tile
from concourse import bacc, mybir
from concourse import bass_utils

N = 2048
D = 512
H = 8
HD = 64
NUM_Z_BINS = 16
MAX_Z = 5.0
SCALE = HD ** -0.5
NCORES = 8
QL = 1024          # query-chunk length (PSUM budget)
QC = N // QL       # 2 query chunks
KT = N // 128      # 16 key tiles

FP32 = mybir.dt.float32
BF16 = mybir.dt.bfloat16
BF16_NP = ml_dtypes.bfloat16

AF = mybir.ActivationFunctionType
OP = mybir.AluOpType

_PROGRAM_CACHE = {}


def _build_program():
    if "nc" in _PROGRAM_CACHE:
        return _PROGRAM_CACHE["nc"]

    nc = bacc.Bacc(
        "TRN2",
        target_bir_lowering=False,
        debug=False,
        enable_asserts=False,
        num_devices=NCORES,
    )

    xT = nc.dram_tensor("xT", [D, N], FP32, kind="ExternalInput").ap()
    wq = nc.dram_tensor("wq", [D, HD], FP32, kind="ExternalInput").ap()
    wk = nc.dram_tensor("wk", [D, HD], FP32, kind="ExternalInput").ap()
    wv = nc.dram_tensor("wv", [D, HD], FP32, kind="ExternalInput").ap()
    wo = nc.dram_tensor("wo", [HD, D], FP32, kind="ExternalInput").ap()
    bq = nc.dram_tensor("bq", [HD], FP32, kind="ExternalInput").ap()
    bk = nc.dram_tensor("bk", [HD], FP32, kind="ExternalInput").ap()
    wt = nc.dram_tensor("wt", [N, N], BF16, kind="ExternalInput").ap()

    ypT = nc.dram_tensor("ypT", [D, N], FP32, kind="ExternalOutput").ap()
    zrow = nc.dram_tensor("zrow", [N], FP32, kind="ExternalOutput").ap()

    with tile.TileContext(nc) as tc:
        with ExitStack() as ctx:
            _emit(ctx, tc, xT, wq, wk, wv, wo, bq, bk, wt, ypT, zrow)
    nc.compile()
    _PROGRAM_CACHE["nc"] = nc
    return nc


def _emit(ctx, tc, xT, wq, wk, wv, wo, bq, bk, wt, ypT, zrow):
    nc = tc.nc
    CH = D // 128  # 4 contraction chunks of the model dim

    singles = ctx.enter_context(tc.tile_pool(name="singles", bufs=1))
    # PSUM budget is 16KB/partition (8 banks). ps_a slots are [128,1024]fp32
    # (4KB/part) shared by the qk/v/s/y phases; ps_o holds the O' accumulators.
    ps_a = ctx.enter_context(tc.tile_pool(name="ps_a", bufs=2, space="PSUM"))
    ps_o = ctx.enter_context(tc.tile_pool(name="ps_o", bufs=2, space="PSUM"))
    wpool = ctx.enter_context(tc.tile_pool(name="wpool", bufs=4))
    epool = ctx.enter_context(tc.tile_pool(name="epool", bufs=3))
    ppool = ctx.enter_context(tc.tile_pool(name="ppool", bufs=3))
    ypool = ctx.enter_context(tc.tile_pool(name="ypool", bufs=4))

    # ---- load constants -------------------------------------------------
    xT_sb = singles.tile([128, CH * N], FP32)          # chunk c at cols [c*N, (c+1)*N)
    for c in range(CH):
        nc.sync.dma_start(
            out=xT_sb[:, c * N:(c + 1) * N],
            in_=xT.rearrange("(c p) n -> c p n", p=128)[c],
        )
    wq_sb = singles.tile([128, CH * HD], FP32)
    wk_sb = singles.tile([128, CH * HD], FP32)
    wv_sb = singles.tile([128, CH * HD], FP32)
    for c in range(CH):
        nc.sync.dma_start(out=wq_sb[:, c * HD:(c + 1) * HD],
                          in_=wq.rearrange("(c p) m -> c p m", p=128)[c])
        nc.sync.dma_start(out=wk_sb[:, c * HD:(c + 1) * HD],
                          in_=wk.rearrange("(c p) m -> c p m", p=128)[c])
        nc.sync.dma_start(out=wv_sb[:, c * HD:(c + 1) * HD],
                          in_=wv.rearrange("(c p) m -> c p m", p=128)[c])
    wo_sb = singles.tile([HD, D], FP32)
    nc.sync.dma_start(out=wo_sb, in_=wo)
    bq_sb = singles.tile([HD, 1], FP32)
    nc.sync.dma_start(out=bq_sb, in_=bq.rearrange("(n a) -> n a", a=1))
    bk_sb = singles.tile([HD, 1], FP32)
    nc.sync.dma_start(out=bk_sb, in_=bk.rearrange("(n a) -> n a", a=1))

    # ---- Q^T, K^T : [64, N] = Wx^T @ x^T, + bias, Q scaled --------------
    qT_sb = singles.tile([HD, N], FP32)
    kT_sb = singles.tile([HD, N], FP32)
    for name, w_sb, out_sb, b_sb, scaled in (
        ("q", wq_sb, qT_sb, bq_sb, True),
        ("k", wk_sb, kT_sb, bk_sb, False),
    ):
        for half in range(N // 1024):
            pt = ps_a.tile([HD, 1024], FP32, tag="big")
            for n in range(2):
                for c in range(CH):
                    col = half * 1024 + n * 512
                    nc.tensor.matmul(
                        pt[:, n * 512:(n + 1) * 512],
                        lhsT=w_sb[:, c * HD:(c + 1) * HD],
                        rhs=xT_sb[:, c * N + col: c * N + col + 512],
                        start=(c == 0),
                        stop=(c == CH - 1),
                    )
            dst = out_sb[:, half * 1024:(half + 1) * 1024]
            if scaled:
                nc.vector.tensor_scalar(dst, pt, b_sb, SCALE, OP.add, OP.mult)
            else:
                nc.vector.tensor_scalar(dst, pt, b_sb, None, OP.add)

    # ---- V' : [k-tile 128, 65] per tile, col 64 = 1.0 -------------------
    v_sb = singles.tile([128, KT * (HD + 1)], BF16)
    nc.vector.memset(v_sb, 1.0)
    vp = ps_a.tile([128, KT * HD], FP32, tag="big")
    for m in range(KT):
        for c in range(CH):
            nc.tensor.matmul(
                vp[:, m * HD:(m + 1) * HD],
                lhsT=xT_sb[:, c * N + m * 128: c * N + (m + 1) * 128],
                rhs=wv_sb[:, c * HD:(c + 1) * HD],
                start=(c == 0),
                stop=(c == CH - 1),
            )
    nc.vector.tensor_copy(
        v_sb.rearrange("p (t c) -> p t c", c=HD + 1)[:, :, 0:HD],
        vp.rearrange("p (t c) -> p t c", c=HD),
    )

    # ---- main loop: S^T -> exp -> *W -> PV ------------------------------
    oT_sb = singles.tile([HD + 1, N], FP32)
    for qc in range(QC):
        ot = ps_o.tile([HD + 1, QL], FP32, tag="ot")
        for t in range(KT):
            st = ps_a.tile([128, QL], FP32, tag="big")
            w_tile = wpool.tile([128, QL], BF16, tag="w")
            nc.sync.dma_start(
                out=w_tile,
                in_=wt[t * 128:(t + 1) * 128, qc * QL:(qc + 1) * QL],
            )
            for n in range(QL // 512):
                nc.tensor.matmul(
                    st[:, n * 512:(n + 1) * 512],
                    lhsT=kT_sb[:, t * 128:(t + 1) * 128],
                    rhs=qT_sb[:, qc * QL + n * 512: qc * QL + (n + 1) * 512],
                    start=True,
                    stop=True,
                )
            e_tile = epool.tile([128, QL], BF16, tag="e")
            nc.scalar.activation(e_tile, st, AF.Exp)
            p_tile = ppool.tile([128, QL], BF16, tag="p")
            nc.vector.scalar_tensor_tensor(
                p_tile, in0=e_tile, scalar=1.0, in1=w_tile,
                op0=OP.mult, op1=OP.mult,
            )
            for n in range(QL // 512):
                nc.tensor.matmul(
                    ot[:, n * 512:(n + 1) * 512],
                    lhsT=v_sb[:, t * (HD + 1):(t + 1) * (HD + 1)],
                    rhs=p_tile[:, n * 512:(n + 1) * 512],
                    start=(t == 0),
                    stop=(t == KT - 1),
                )
        nc.vector.tensor_copy(oT_sb[:, qc * QL:(qc + 1) * QL], ot)

    # ---- outputs --------------------------------------------------------
    nc.sync.dma_start(out=zrow.rearrange("(a n) -> a n", a=1), in_=oT_sb[HD:HD + 1, :])

    for m in range(D // 128):
        for n in range(N // 512):
            yt = ps_a.tile([128, 512], FP32, tag="big")
            nc.tensor.matmul(
                yt,
                lhsT=wo_sb[:, m * 128:(m + 1) * 128],
                rhs=oT_sb[0:HD, n * 512:(n + 1) * 512],
                start=True,
                stop=True,
            )
            y_sb = ypool.tile([128, 512], FP32, tag="ysb")
            nc.any.tensor_copy(y_sb, yt)
            nc.sync.dma_start(
                out=ypT[m * 128:(m + 1) * 128, n * 512:(n + 1) * 512],
                in_=y_sb,
            )


def _install_ntff_hook():
    """Recreate the missing ``antenv.axon_hooks`` module so that
    run_bass_kernel_spmd(trace=True) can capture NTFF profiles via the
    libaxon_pjrt.so ctypes hook (see trn_agent_boot.trn_boot)."""
    import sys
    import types

    try:
        import antenv.axon_hooks  # noqa: F401
        return
    except ImportError:
        pass
    import antenv
    from trn_agent_boot.trn_boot import _ntff_profile_via_ctypes

    mod = types.ModuleType("antenv.axon_hooks")
    mod._hook = _ntff_profile_via_ctypes("/opt/axon/libaxon_pjrt.so")
    mod.set_axon_ntff_profile_hook = lambda h: setattr(mod, "_hook", h)
    mod.get_axon_ntff_profile_hook = lambda: mod._hook
    sys.modules["antenv.axon_hooks"] = mod
    antenv.axon_hooks = mod
    # keep profile artifacts local; the sandbox has no bucket access
    bass_utils.upload_artifacts = lambda tmpdir: tmpdir


def kernel(x, z_matrix, Wq, bq, Wk, bk, Wv, bv, Wo, bo, z_table, _trace=False):
    if _trace:
        _install_ntff_hook()
    x = np.ascontiguousarray(np.asarray(x, dtype=np.float32))
    z_matrix = np.asarray(z_matrix, dtype=np.float32)
    Wq = np.asarray(Wq, dtype=np.float32)
    Wk = np.asarray(Wk, dtype=np.float32)
    Wv = np.asarray(Wv, dtype=np.float32)
    Wo = np.asarray(Wo, dtype=np.float32)
    bq = np.asarray(bq, dtype=np.float32)
    bk = np.asarray(bk, dtype=np.float32)
    bv = np.asarray(bv, dtype=np.float32)
    bo = np.asarray(bo, dtype=np.float32)
    z_table = np.asarray(z_table, dtype=np.float32)

    nc = _build_program()

    xT = np.ascontiguousarray(x.T)
    binsT = np.clip(
        np.floor(z_matrix.T / MAX_Z * NUM_Z_BINS).astype(np.int32), 0, NUM_Z_BINS - 1
    )
    exp_tab = np.exp(z_table)  # [16, H] fp32

    in_maps = []
    for h in range(NCORES):
        sl = slice(h * HD, (h + 1) * HD)
        wt_h = exp_tab[:, h][binsT].astype(BF16_NP)  # [key, query] layout
        in_maps.append({
            "xT": xT,
            "wq": np.ascontiguousarray(Wq[:, sl]),
            "wk": np.ascontiguousarray(Wk[:, sl]),
            "wv": np.ascontiguousarray(Wv[:, sl]),
            "wo": np.ascontiguousarray(Wo[sl, :]),
            "bq": np.ascontiguousarray(bq[sl]),
            "bk": np.ascontiguousarray(bk[sl]),
            "wt": wt_h,
        })

    res = bass_utils.run_bass_kernel_spmd(
        nc, in_maps, core_ids=list(range(NCORES)), trace=_trace,
    )

    acc = np.zeros((D, N), dtype=np.float64)
    for h in range(NCORES):
        ypT_h = res.results[h]["ypT"].astype(np.float64)
        z_h = res.results[h]["zrow"].astype(np.float64)
        acc += ypT_h / z_h[None, :]
    out = acc.T + (bv @ Wo)[None, :] + bo[None, :]
    out_f32 = out.astype(np.float32)
    if _trace:
        return out_f32, res
    return out_f32
